# revision 4
# baseline (speedup 1.0000x reference)
"""MoE layer (flux AG-scatter / gather-RS) Trainium2 kernel.

Problem: NTOK=8192, HIDDEN=2048, FFN=2048, NEXP=8, TOPK=2,
scatter_index = arange(M).reshape(NTOK, 2)  (per spec fill="arange").

With arange routing, token t's two slots (rows 2t, 2t+1) both land in
expert t//1024's contiguous block, so the layer reduces to: for expert
e, out[t] = 2 * gelu(x_t @ w1[e].T) @ w2[e].T for t in [1024e, 1024e+1024).
Routing is block-diagonal => pure expert-parallelism, no collectives.
The x2 is folded into w2 on the host (exact: power-of-two scale).

Each of the 8 cores runs one expert: two [1024x2048]@[2048x2048] GEMMs
with exact-gelu in between, in f32r (TF32: fp32 with 11-bit mantissa)
which streams the PE at bf16 rate. Host pre-rounds inputs to f32r and
pre-transposes/pre-tiles all operands so every DMA is contiguous.

A general fallback (any scatter_index permutation) does host-side
scatter/combine and runs the same device program over 16 virtual
shards in two launches.
"""

import sys

sys.path.insert(0, "/opt/trn_rl_repo")

import numpy as np

import concourse.bass as bass
import concourse.mybir as mybir
import concourse.tile as tile
from concourse.bass_utils import run_bass_kernel_spmd

f32 = mybir.dt.float32
f32r = mybir.dt.float32r
ACTF = mybir.ActivationFunctionType

NTOK, HIDDEN, FFN, NEXP, TOPK = 8192, 2048, 2048, 8, 2
M = NTOK * TOPK
CAP = M // NEXP
NCORE = 8
P = 128
C = 1024  # tokens per core (fast path: unique tokens per expert)
KH = HIDDEN // P  # 16 contraction chunks for gemm1
NF = FFN // P  # 16 f-row tiles of G^T
NH = HIDDEN // P  # 16 h-row tiles of Y^T
CT = C // 512  # c-column tiles (moving dim 512)


def _round_tf32(x: np.ndarray) -> np.ndarray:
    """Round fp32 to f32r (TF32): 11-bit mantissa, RNE, low 12 bits zero."""
    u = np.ascontiguousarray(x, dtype=np.float32).view(np.uint32)
    lsb = (u >> 12) & 1
    r = (u + np.uint32(0x7FF) + lsb) & np.uint32(0xFFFFF000)
    return r.view(np.float32)


def _split_sync_waits(nc, cap: int = 1) -> int:
    """Walrus codegen structs have tiny sync-wait capacity (fused-LDW
    Matmult: 1). Hoist all-but-`cap` waits of each instruction onto
    preceding same-engine NoOps (one wait each); sequencers execute
    waits in program order, so semantics are unchanged."""
    n = 0
    for fn in nc.m.functions:
        for blk in fn.blocks:
            out = []
            for inst in blk.instructions:
                si = inst.sync_info
                waits = list(si.on_wait) if si is not None else []
                if len(waits) > cap:
                    excess, keep = waits[:-cap], waits[-cap:]
                    for w in excess:
                        nop = mybir.InstNoOp(
                            name=f"{inst.name}-wsplit{n}",
                            engine=inst.engine,
                            sync_info=mybir.SyncInfo(on_wait=[w], on_update=[]),
                            bass_nofuse=True,
                        )
                        nc.register_instruction(nop, overwrite=True)
                        out.append(nop)
                        n += 1
                    inst.sync_info = mybir.SyncInfo(
                        on_wait=keep, on_update=list(si.on_update)
                    )
                out.append(inst)
            blk.instructions = out
    return n


def build_program():
    """One expert-shard: yT = (gelu(x @ w1.T) @ w2s.T).T, all operands
    pre-tiled on host. Shapes (per core):
      xT  [KH, P, C]   f32r   xT[k,p,c]  = x[c, k*128+p]
      w1p [NF, P, KH*P] f32r  w1p[f,p,k*128+j] = w1[f*128+j, k*128+p]
      w2p [NH, P, NF*P] f32r  w2p[h,p,f*128+j] = w2s[h*128+j, f*128+p]
      yT  [NH, P, C]   f32    yT[h,p,c]  = y[c, h*128+p]
    """
    nc = bass.Bass()
    xT = nc.declare_dram_parameter("xT", [KH, P, C], f32r, isOutput=False)
    w1p = nc.declare_dram_parameter("w1p", [NF, P, KH * P], f32r, isOutput=False)
    w2p = nc.declare_dram_parameter("w2p", [NH, P, NF * P], f32r, isOutput=False)
    yT = nc.declare_dram_parameter("yT", [NH, P, C], f32, isOutput=True)

    with tile.TileContext(nc) as tc:
        with (
            tc.tile_pool(name="xp", bufs=1) as xpool,
            tc.tile_pool(name="gp", bufs=1) as gpool,
            tc.tile_pool(name="w1s", bufs=2) as w1pool,
            tc.tile_pool(name="w2s", bufs=2) as w2pool,
            tc.tile_pool(name="yp", bufs=3) as ypool,
            tc.tile_pool(name="ps", bufs=8, space="PSUM") as psum,
        ):
            x_tiles = []
            for ki in range(KH):
                t = xpool.tile([P, C], f32r, tag=f"x{ki}", name=f"x{ki}")
                nc.sync.dma_start(out=t, in_=xT[ki])
                x_tiles.append(t)

            g_tiles = [
                gpool.tile([P, C], f32r, tag=f"g{fi}", name=f"g{fi}")
                for fi in range(NF)
            ]

            # gemm1 + gelu: G^T tiles [f 128, c 512]
            for fi in range(NF):
                w1t = w1pool.tile([P, KH * P], f32r, tag="w1t", name=f"w1t{fi}")
                nc.sync.dma_start(out=w1t, in_=w1p[fi])
                for ci in range(CT):
                    ps = psum.tile([P, 512], f32, tag="ps", name="ps")
                    for ki in range(KH):
                        nc.tensor.matmul(
                            ps,
                            lhsT=w1t[:, ki * P : (ki + 1) * P],
                            rhs=x_tiles[ki][:, ci * 512 : (ci + 1) * 512],
                            start=(ki == 0),
                            stop=(ki == KH - 1),
                        )
                    nc.scalar.activation(
                        out=g_tiles[fi][:, ci * 512 : (ci + 1) * 512],
                        in_=ps,
                        func=ACTF.Gelu,
                    )

            # gemm2: Y^T tiles [h 128, c 512], DVE drains PSUM
            for hi in range(NH):
                w2t = w2pool.tile([P, NF * P], f32r, tag="w2t", name=f"w2t{hi}")
                nc.sync.dma_start(out=w2t, in_=w2p[hi])
                yt = ypool.tile([P, C], f32, tag="yt", name=f"yt{hi}")
                for ci in range(CT):
                    ps = psum.tile([P, 512], f32, tag="ps", name="ps")
                    for fi in range(NF):
                        nc.tensor.matmul(
                            ps,
                            lhsT=w2t[:, fi * P : (fi + 1) * P],
                            rhs=g_tiles[fi][:, ci * 512 : (ci + 1) * 512],
                            start=(fi == 0),
                            stop=(fi == NF - 1),
                        )
                    nc.vector.tensor_copy(yt[:, ci * 512 : (ci + 1) * 512], ps)
                nc.sync.dma_start(out=yT[hi], in_=yt)

    _split_sync_waits(nc)
    return nc


def _pack_x(x_block: np.ndarray) -> np.ndarray:
    """[C, HIDDEN] -> xT [KH, P, C] f32r-rounded."""
    xt = np.ascontiguousarray(x_block.T)  # [HIDDEN, C]
    return _round_tf32(xt).reshape(KH, P, C)


def _pack_w(w: np.ndarray) -> np.ndarray:
    """[out=2048, in=2048] -> [16, P, 16*P] with [oi, p, ki*128+j] =
    w[oi*128+j, ki*128+p] (lhsT tiles along the free dim)."""
    t = w.reshape(NF, P, KH, P).transpose(0, 3, 2, 1)  # [oi, p, ki, j]
    return _round_tf32(np.ascontiguousarray(t)).reshape(NF, P, KH * P)


def _unpack_y(yT: np.ndarray) -> np.ndarray:
    """yT [NH, P, C] -> y [C, HIDDEN]."""
    return yT.reshape(HIDDEN, C).T


_CACHED_NC = None


def _get_nc():
    global _CACHED_NC
    if _CACHED_NC is None:
        _CACHED_NC = build_program()
    return _CACHED_NC


def _run_shards(x_blocks, w1_list, w2_list):
    """Run len(x_blocks) virtual shards (multiples of NCORE per launch)."""
    nc = _get_nc()
    n = len(x_blocks)
    outs = []
    for base in range(0, n, NCORE):
        in_maps = []
        for j in range(base, min(base + NCORE, n)):
            in_maps.append(
                {
                    "xT": _pack_x(x_blocks[j]),
                    "w1p": _pack_w(w1_list[j]),
                    "w2p": _pack_w(w2_list[j]),
                }
            )
        res = run_bass_kernel_spmd(nc, in_maps, list(range(len(in_maps))))
        outs.extend(_unpack_y(r["yT"]) for r in res.results)
    return outs


def kernel(inputs_shard, w1, w2, scatter_index):
    x = np.ascontiguousarray(inputs_shard, dtype=np.float32)
    w1 = np.asarray(w1, dtype=np.float32)
    w2 = np.asarray(w2, dtype=np.float32)
    si = np.asarray(scatter_index)

    fast = np.array_equal(si.ravel(), np.arange(M, dtype=np.int64))
    out = np.empty((NTOK, HIDDEN), dtype=np.float32)

    if fast:
        # expert e <- tokens [1024e, 1024e+1024), result doubled via w2
        xb = [x[e * C : (e + 1) * C] for e in range(NEXP)]
        w1l = [w1[e] for e in range(NEXP)]
        w2l = [2.0 * w2[e] for e in range(NEXP)]
        ys = _run_shards(xb, w1l, w2l)
        for e in range(NEXP):
            out[e * C : (e + 1) * C] = ys[e]
        return out

    # general path: host scatter -> 16 virtual shards -> host combine
    x_rep = np.repeat(x, TOPK, axis=0)
    x_scat = np.zeros((M, HIDDEN), dtype=np.float32)
    x_scat[si.ravel()] = x_rep
    nsh = M // C  # 16
    xb = [x_scat[j * C : (j + 1) * C] for j in range(nsh)]
    w1l = [w1[j * C // CAP] for j in range(nsh)]
    w2l = [w2[j * C // CAP] for j in range(nsh)]
    ys = _run_shards(xb, w1l, w2l)
    y_scat = np.concatenate(ys, axis=0)
    out[:] = y_scat[si].sum(axis=1)
    return out


# revision 11
# speedup vs baseline: 23.2435x; 23.2435x over previous
"""MoE layer (flux AG-scatter / gather-RS) Trainium2 kernel.

Problem: NTOK=8192, HIDDEN=2048, FFN=2048, NEXP=8, TOPK=2,
scatter_index = arange(M).reshape(NTOK, 2)  (per spec fill="arange").

With arange routing, token t's two slots (rows 2t, 2t+1) both land in
expert t//1024's contiguous block, so the layer reduces to: for expert
e, out[t] = 2 * gelu(x_t @ w1[e].T) @ w2[e].T for t in [1024e, 1024e+1024).
Routing is block-diagonal => pure expert-parallelism, no collectives.
The x2 is folded into w2 on the host (exact: power-of-two scale).

Each of the 8 cores runs one expert: two [1024x2048]@[2048x2048] GEMMs
with exact-gelu in between, in f32r (TF32: fp32 with 11-bit mantissa)
which streams the PE at bf16 rate. Host pre-rounds inputs to f32r and
pre-transposes/pre-tiles all operands so every DMA is contiguous.

A general fallback (any scatter_index permutation) does host-side
scatter/combine and runs the same device program over 16 virtual
shards in two launches.
"""

import sys

sys.path.insert(0, "/opt/trn_rl_repo")

import numpy as np

import concourse.bass as bass
import concourse.mybir as mybir
import concourse.tile as tile
from concourse.bass_utils import run_bass_kernel_spmd

f32 = mybir.dt.float32
f32r = mybir.dt.float32r
ACTF = mybir.ActivationFunctionType

NTOK, HIDDEN, FFN, NEXP, TOPK = 8192, 2048, 2048, 8, 2
M = NTOK * TOPK
CAP = M // NEXP
NCORE = 8
P = 128
C = 1024  # tokens per core (fast path: unique tokens per expert)
KH = HIDDEN // P  # 16 contraction chunks for gemm1
NF = FFN // P  # 16 f-row tiles of G^T
NH = HIDDEN // P  # 16 h-row tiles of Y^T
CT = C // 512  # c-column tiles (moving dim 512)


def _round_tf32(x: np.ndarray) -> np.ndarray:
    """Round fp32 to f32r (TF32): 11-bit mantissa, RNE, low 12 bits zero."""
    u = np.ascontiguousarray(x, dtype=np.float32).view(np.uint32)
    lsb = (u >> 12) & 1
    r = (u + np.uint32(0x7FF) + lsb) & np.uint32(0xFFFFF000)
    return r.view(np.float32)


def _split_sync_waits(nc, cap: int = 1) -> int:
    """Walrus codegen structs have tiny sync-wait capacity (fused-LDW
    Matmult: 1). Hoist all-but-`cap` waits of each instruction onto
    preceding same-engine NoOps (one wait each); sequencers execute
    waits in program order, so semantics are unchanged."""
    n = 0
    for fn in nc.m.functions:
        for blk in fn.blocks:
            out = []
            for inst in blk.instructions:
                si = inst.sync_info
                waits = list(si.on_wait) if si is not None else []
                if len(waits) > cap:
                    excess, keep = waits[:-cap], waits[-cap:]
                    for w in excess:
                        nop = mybir.InstNoOp(
                            name=f"{inst.name}-wsplit{n}",
                            engine=inst.engine,
                            sync_info=mybir.SyncInfo(on_wait=[w], on_update=[]),
                            bass_nofuse=True,
                        )
                        nc.register_instruction(nop, overwrite=True)
                        out.append(nop)
                        n += 1
                    inst.sync_info = mybir.SyncInfo(
                        on_wait=keep, on_update=list(si.on_update)
                    )
                out.append(inst)
            blk.instructions = out
    return n


def build_program(repeat: int = 1, warmup_mms: int = 250):
    """One expert-shard: yT = (gelu(x @ w1.T) @ w2s.T).T, all operands
    pre-tiled on host. Shapes (per core):
      xT  [KH, P, C]   f32r   xT[k,p,c]  = x[c, k*128+p]
      w1p [NF, P, KH*P] f32r  w1p[f,p,k*128+j] = w1[f*128+j, k*128+p]
      w2p [NH, P, NF*P] f32r  w2p[h,p,f*128+j] = w2s[h*128+j, f*128+p]
      yT  [NH, P, C]   f32    yT[h,p,c]  = y[c, h*128+p]

    repeat > 1 re-runs the identical body into the same buffers (for
    overhead-cancelling slope timing); results are unchanged.
    """
    nc = bass.Bass()
    xT = nc.declare_dram_parameter("xT", [KH, P, C], f32r, isOutput=False)
    w1p = nc.declare_dram_parameter("w1p", [NF, P, KH * P], f32r, isOutput=False)
    w2p = nc.declare_dram_parameter("w2p", [NH, P, NF * P], f32r, isOutput=False)
    yT = nc.declare_dram_parameter("yT", [NH, P, C], f32, isOutput=True)

    with tile.TileContext(nc) as tc:
        with (
            tc.tile_pool(name="xp", bufs=1) as xpool,
            tc.tile_pool(name="gp", bufs=1) as gpool,
            tc.tile_pool(name="w1s", bufs=2) as w1pool,
            tc.tile_pool(name="w2s", bufs=2) as w2pool,
            tc.tile_pool(name="yp", bufs=3) as ypool,
            tc.tile_pool(name="ps", bufs=8, space="PSUM") as psum,
        ):
            g_tiles = [
                gpool.tile([P, C], f32r, tag=f"g{fi}", name=f"g{fi}")
                for fi in range(NF)
            ]
            x_tiles = [
                xpool.tile([P, C], f32r, tag=f"x{ki}", name=f"x{ki}")
                for ki in range(KH)
            ]

            if warmup_mms:
                # Keep the PE busy while the first x/w1 DMAs stream in, so
                # the HAM clock gate is at 8/8 when real matmuls start.
                # bf16 64-row matmuls are ~27ns each: fine-grained filler.
                bf16 = mybir.dt.bfloat16
                wut = xpool.tile([P, 64], bf16, tag="wut", name="wut")
                nc.gpsimd.memset(wut, 1.0)
                wups = psum.tile([P, 512], f32, tag="ps", name="ps")
                for _ in range(warmup_mms):
                    nc.tensor.matmul(
                        wups[:64, :64], lhsT=wut, rhs=wut, start=True, stop=True
                    )

            for _ in range(repeat):
                # first w1 block before the bulk of x, then x in ci-halves so
                # the first gemm1 group's operands land ASAP
                w1_tiles = [
                    w1pool.tile([P, KH * P], f32r, tag="w1t", name=f"w1t{fi}")
                    for fi in range(2)
                ]
                nc.sync.dma_start(out=w1_tiles[0], in_=w1p[0])
                for ci in range(CT):
                    for ki in range(KH):
                        nc.sync.dma_start(
                            out=x_tiles[ki][:, ci * 512 : (ci + 1) * 512],
                            in_=xT[ki][:, ci * 512 : (ci + 1) * 512],
                        )

                # gemm1 + gelu: G^T tiles [f 128, c 512]
                for fi in range(NF):
                    if fi < 2:
                        w1t = w1_tiles[fi]
                        if fi == 1:
                            nc.sync.dma_start(out=w1t, in_=w1p[1])
                    else:
                        w1t = w1pool.tile(
                            [P, KH * P], f32r, tag="w1t", name=f"w1t{fi}"
                        )
                        nc.sync.dma_start(out=w1t, in_=w1p[fi])
                    for ci in range(CT):
                        ps = psum.tile([P, 512], f32, tag="ps", name="ps")
                        for ki in range(KH):
                            nc.tensor.matmul(
                                ps,
                                lhsT=w1t[:, ki * P : (ki + 1) * P],
                                rhs=x_tiles[ki][:, ci * 512 : (ci + 1) * 512],
                                start=(ki == 0),
                                stop=(ki == KH - 1),
                            )
                        nc.scalar.activation(
                            out=g_tiles[fi][:, ci * 512 : (ci + 1) * 512],
                            in_=ps,
                            func=ACTF.Gelu,
                        )

                # gemm2: Y^T tiles [h 128, c 512], DVE drains PSUM
                for hi in range(NH):
                    w2t = w2pool.tile([P, NF * P], f32r, tag="w2t", name=f"w2t{hi}")
                    nc.sync.dma_start(out=w2t, in_=w2p[hi])
                    yt = ypool.tile([P, C], f32, tag="yt", name=f"yt{hi}")
                    for ci in range(CT):
                        ps = psum.tile([P, 512], f32, tag="ps", name="ps")
                        for fi in range(NF):
                            nc.tensor.matmul(
                                ps,
                                lhsT=w2t[:, fi * P : (fi + 1) * P],
                                rhs=g_tiles[fi][:, ci * 512 : (ci + 1) * 512],
                                start=(fi == 0),
                                stop=(fi == NF - 1),
                            )
                        nc.vector.tensor_copy(yt[:, ci * 512 : (ci + 1) * 512], ps)
                    nc.sync.dma_start(out=yT[hi], in_=yt)

    _split_sync_waits(nc)
    return nc


def _pack_x(x_block: np.ndarray) -> np.ndarray:
    """[C, HIDDEN] -> xT [KH, P, C] f32r-rounded."""
    xt = np.ascontiguousarray(x_block.T)  # [HIDDEN, C]
    return _round_tf32(xt).reshape(KH, P, C)


def _pack_w(w: np.ndarray) -> np.ndarray:
    """[out=2048, in=2048] -> [16, P, 16*P] with [oi, p, ki*128+j] =
    w[oi*128+j, ki*128+p] (lhsT tiles along the free dim)."""
    t = w.reshape(NF, P, KH, P).transpose(0, 3, 2, 1)  # [oi, p, ki, j]
    return _round_tf32(np.ascontiguousarray(t)).reshape(NF, P, KH * P)


def _unpack_y(yT: np.ndarray) -> np.ndarray:
    """yT [NH, P, C] -> y [C, HIDDEN]."""
    return yT.reshape(HIDDEN, C).T


_CACHED_NC = None


def _get_nc():
    global _CACHED_NC
    if _CACHED_NC is None:
        _CACHED_NC = build_program()
    return _CACHED_NC


def _run_shards(x_blocks, w1_list, w2_list):
    """Run len(x_blocks) virtual shards (multiples of NCORE per launch)."""
    nc = _get_nc()
    n = len(x_blocks)
    outs = []
    for base in range(0, n, NCORE):
        in_maps = []
        for j in range(base, min(base + NCORE, n)):
            in_maps.append(
                {
                    "xT": _pack_x(x_blocks[j]),
                    "w1p": _pack_w(w1_list[j]),
                    "w2p": _pack_w(w2_list[j]),
                }
            )
        res = run_bass_kernel_spmd(nc, in_maps, list(range(len(in_maps))))
        outs.extend(_unpack_y(r["yT"]) for r in res.results)
    return outs


def kernel(inputs_shard, w1, w2, scatter_index):
    x = np.ascontiguousarray(inputs_shard, dtype=np.float32)
    w1 = np.asarray(w1, dtype=np.float32)
    w2 = np.asarray(w2, dtype=np.float32)
    si = np.asarray(scatter_index)

    fast = np.array_equal(si.ravel(), np.arange(M, dtype=np.int64))
    out = np.empty((NTOK, HIDDEN), dtype=np.float32)

    if fast:
        # expert e <- tokens [1024e, 1024e+1024), result doubled via w2
        xb = [x[e * C : (e + 1) * C] for e in range(NEXP)]
        w1l = [w1[e] for e in range(NEXP)]
        w2l = [2.0 * w2[e] for e in range(NEXP)]
        ys = _run_shards(xb, w1l, w2l)
        for e in range(NEXP):
            out[e * C : (e + 1) * C] = ys[e]
        return out

    # general path: host scatter -> 16 virtual shards -> host combine
    x_rep = np.repeat(x, TOPK, axis=0)
    x_scat = np.zeros((M, HIDDEN), dtype=np.float32)
    x_scat[si.ravel()] = x_rep
    nsh = M // C  # 16
    xb = [x_scat[j * C : (j + 1) * C] for j in range(nsh)]
    w1l = [w1[j * C // CAP] for j in range(nsh)]
    w2l = [w2[j * C // CAP] for j in range(nsh)]
    ys = _run_shards(xb, w1l, w2l)
    y_scat = np.concatenate(ys, axis=0)
    out[:] = y_scat[si].sum(axis=1)
    return out
